# revision 7
# baseline (speedup 1.0000x reference)
"""Differential attention (DIFF Transformer layer) on 8 Trainium2 NeuronCores.

Sharding: tensor-parallel over heads x data-parallel over batch.
Core c (0..7) handles batch b = c//4 and the head-quad qd = c%4
(heads 4*qd .. 4*qd+3 of 16, BOTH score groups). Each core computes its
heads' q/k/v projections, causal softmax attention for both groups,
the differential combine (a1@v1 - lam*a2@v2)*(1-lam_init), and a
row-parallel partial of the output projection. The host sums the 4
partial outputs per batch (the unshard step of row-parallel TP).

Kernel structure per core (all matmuls on PE, fp32r for the large
512-wide-moving matmuls, bf16 for the A@V accumulation):
  0. x_b -> x^T in SBUF via PE transposes                  [128,8,2048]
  1. q^T, k^T = W^T @ x^T (per-head-transposed layouts), v natural
  2. flash-style causal attention per (head, group):
       s^T[kpos,q] = K^T.T @ Q^T ; A = exp(s/8) (no max needed: |s|<3)
       diagonal blocks masked with affine_select; o[q,:] accumulated
       in PSUM via A^T-chunk-stationary matmuls against V'=[V|1]
       (the ones column yields softmax row sums for free)
  3. normalize by row sums, combine groups, transpose o, o @ Wo slice
"""

import os

import numpy as np

import concourse.bass as bass
import concourse.mybir as mybir
import concourse.tile as tile
from concourse.bass_utils import run_bass_kernel_spmd
from concourse.masks import make_identity
from concourse.vector_clock import ScopedClock
from contextlib import ExitStack


_MAX_WAITS = 1  # walrus setupSyncWait caps sem-waits per instruction


def _spill_excess_waits(nc):
    """This walrus build rejects instructions carrying more than a couple
    of sem-waits (setupSyncWait: 'Too many sync wait commands'). Move the
    excess onto same-engine NoOps inserted just before the instruction —
    the engine blocks on the NoOps' waits first, so semantics match."""
    idx = 0
    for f in nc.m.functions:
        for bb in f.blocks:
            new = []
            changed = False
            for inst in bb.instructions:
                si = getattr(inst, "sync_info", None)
                waits = list(si.on_wait) if si is not None and si.on_wait else []
                if (
                    len(waits) > _MAX_WAITS
                    and inst.engine != mybir.EngineType.Unassigned
                ):
                    changed = True
                    excess = waits[: -_MAX_WAITS]
                    for j in range(0, len(excess), _MAX_WAITS):
                        nop = mybir.InstNoOp(
                            name=f"wspill-{idx}",
                            bass_nofuse=True,
                            sync_info=mybir.SyncInfo(
                                on_wait=excess[j : j + _MAX_WAITS], on_update=[]
                            ),
                        )
                        idx += 1
                        nop.engine = inst.engine
                        nc.register_instruction(nop)
                        new.append(nop)
                    si.on_wait = waits[-_MAX_WAITS:]
                new.append(inst)
            if changed:
                bb.instructions = new


_orig_drain_and_barrier = tile.TileContext._drain_and_barrier


def _drain_barrier_and_spill(self, tick_clock, wait_clock):
    _orig_drain_and_barrier(self, tick_clock, wait_clock)
    _spill_excess_waits(self.nc)


tile.TileContext._drain_and_barrier = _drain_barrier_and_spill

P = 128
S = 2048
D = 1024
DH = 64
NH_TOT = 16
NHC = 4  # heads per core
NG = 2  # score groups
LAMBDA_INIT = 0.8
NCORES = 8

F32 = mybir.dt.float32
F32R = mybir.dt.float32r
BF16 = mybir.dt.bfloat16
EXP = mybir.ActivationFunctionType.Exp
MULT = mybir.AluOpType.mult
IS_GE = mybir.AluOpType.is_ge

TOKC = S // P  # 16 token chunks
DC = D // P  # 8 d_model chunks
QB = 512  # q block width for score matmuls
NQB = S // QB  # 4
WCOLS = NHC * NG * DH  # 512 projection cols per core
OROWS = NHC * DH  # 256 o_proj rows per core

LAST_RESULT = None  # test harness reads exec_time_ns from here


def _r(ap):
    return ap.bitcast(F32R)


def build_program(c1: float, c2: float) -> bass.Bass:
    """c1 = (1-lambda_init), c2 = (1-lambda_init)*lambda — baked immediates."""
    nc = bass.Bass("TRN2", target_bir_lowering=False, debug=False)

    xb = nc.dram_tensor("xb", [S, D], F32, kind="ExternalInput").ap()
    wq = nc.dram_tensor("wq", [D, WCOLS], F32, kind="ExternalInput").ap()
    wk = nc.dram_tensor("wk", [D, WCOLS], F32, kind="ExternalInput").ap()
    wv = nc.dram_tensor("wv", [D, WCOLS], F32, kind="ExternalInput").ap()
    wo = nc.dram_tensor("wo", [OROWS, D], F32, kind="ExternalInput").ap()
    out = nc.dram_tensor("out", [S, D], F32, kind="ExternalOutput").ap()

    with tile.TileContext(nc) as tc, ExitStack() as es:
        const_pool = es.enter_context(tc.tile_pool(name="const", bufs=1))
        ident = const_pool.tile([P, P], F32)
        make_identity(nc, ident)

        persist = es.enter_context(tc.tile_pool(name="persist", bufs=1))
        # q^T/k^T: chunk hh holds head hh, group 0 rows 0:64, group 1 rows 64:128
        qT = persist.tile([P, NHC, S], F32R)
        kT = persist.tile([P, NHC, S], F32R)
        # v natural + ones column: [kpos-in-chunk, chunk, strip 2*hh+g, dh+1]
        vS = persist.tile([P, TOKC, NHC * NG, DH + 1], BF16)
        nc.vector.memset(vS[:, :, :, DH], 1.0)
        # diff-combined attention out, natural layout [q-in-chunk, chunk, odim]
        o_d = persist.tile([P, TOKC, OROWS], F32)

        # ---- phase 0+1: x^T, projections ----
        with ExitStack() as ph1:
            xT_pool = ph1.enter_context(tc.tile_pool(name="xT", bufs=1))
            xT = xT_pool.tile([P, DC, S], F32R)
            with ExitStack() as ph0:
                xs_pool = ph0.enter_context(tc.tile_pool(name="xs", bufs=3))
                tp_psum = ph0.enter_context(
                    tc.tile_pool(name="tpp", bufs=4, space="PSUM")
                )
                for t in range(TOKC):
                    xstage = xs_pool.tile([P, D], F32, tag="xs")
                    nc.sync.dma_start(xstage[:], xb[t * P : (t + 1) * P, :])
                    for dc in range(DC):
                        pt = tp_psum.tile([P, P], F32, tag="tp")
                        nc.tensor.transpose(
                            pt[:], xstage[:, dc * P : (dc + 1) * P], ident[:]
                        )
                        nc.vector.tensor_copy(xT[:, dc, t * P : (t + 1) * P], pt[:])

            wst_pool = ph1.enter_context(tc.tile_pool(name="wst", bufs=3))
            wr_pool = ph1.enter_context(tc.tile_pool(name="wr", bufs=2))
            pr_psum = ph1.enter_context(tc.tile_pool(name="prp", bufs=8, space="PSUM"))

            def load_w_f32r(wdram):
                """DMA one W then round it to f32r via DVE (fp32r matmul
                operands must come from an instruction with f32r output)."""
                w_r = wr_pool.tile([P, DC, WCOLS], F32R, tag="wr", name="wr")
                for dc in range(DC):
                    wst = wst_pool.tile([P, WCOLS], F32, tag="wst", name="wst")
                    nc.sync.dma_start(wst[:], wdram[dc * P : (dc + 1) * P, :])
                    nc.vector.tensor_copy(w_r[:, dc, :], wst[:])
                return w_r

            # q^T / k^T: out[dims 128, tok 512] = Wslice.T @ x^T
            for wdram, dstT in ((wq, qT), (wk, kT)):
                w_r = load_w_f32r(wdram)
                for nb in range(NQB):
                    psums = [
                        pr_psum.tile([P, QB], F32, tag="prps", name="prps") for _ in range(NHC)
                    ]
                    for dc in range(DC):
                        for mc in range(NHC):
                            nc.tensor.matmul(
                                psums[mc][:],
                                lhsT=w_r[:, dc, mc * P : (mc + 1) * P],
                                rhs=xT[:, dc, nb * QB : (nb + 1) * QB],
                                start=(dc == 0),
                                stop=(dc == DC - 1),
                            )
                    for mc in range(NHC):
                        nc.vector.tensor_copy(
                            dstT[:, mc, nb * QB : (nb + 1) * QB], psums[mc][:]
                        )

            # v: out[tok 128, strips 512] = x^T-chunk.T @ Wv
            w_r = load_w_f32r(wv)
            for tg in range(TOKC // 4):
                vps = [pr_psum.tile([P, WCOLS], F32, tag="prps", name="prps") for _ in range(4)]
                for dc in range(DC):
                    for i in range(4):
                        t = tg * 4 + i
                        nc.tensor.matmul(
                            vps[i][:],
                            lhsT=xT[:, dc, t * P : (t + 1) * P],
                            rhs=w_r[:, dc, :],
                            start=(dc == 0),
                            stop=(dc == DC - 1),
                        )
                for i in range(4):
                    t = tg * 4 + i
                    nc.vector.tensor_copy(
                        vS[:, t, :, 0:DH],
                        vps[i][:].rearrange("p (s d) -> p s d", s=NHC * NG),
                    )

        # ---- phase 2: attention ----
        with ExitStack() as ph2:
            a_pool = ph2.enter_context(tc.tile_pool(name="a", bufs=4))
            s_psum = ph2.enter_context(tc.tile_pool(name="sps", bufs=2, space="PSUM"))
            o_psum = ph2.enter_context(tc.tile_pool(name="ops", bufs=4, space="PSUM"))
            nrm_pool = ph2.enter_context(tc.tile_pool(name="nrm", bufs=4))

            for qb in range(NQB):
                for hh in range(NHC):
                    og = [o_psum.tile([P, 4, DH + 1], F32, tag="og", name="og") for _ in range(NG)]
                    for g in range(NG):
                        row0 = g * DH
                        strip = 2 * hh + g
                        for kc in range(4 * (qb + 1)):
                            sp = s_psum.tile([P, QB], F32, tag="sp")
                            nc.tensor.matmul(
                                sp[:],
                                lhsT=kT[row0 : row0 + DH, hh, kc * P : (kc + 1) * P],
                                rhs=qT[row0 : row0 + DH, hh, qb * QB : (qb + 1) * QB],
                                start=True,
                                stop=True,
                            )
                            at = a_pool.tile([P, QB], BF16, tag="at")
                            r = (kc - 4 * qb) * P
                            if r >= 0:  # diagonal chunk: cols < r are unused
                                nc.scalar.activation(
                                    at[:, r:QB], sp[:, r:QB], EXP, scale=0.125
                                )
                                # band [r, r+128): keep where col >= row
                                nc.gpsimd.affine_select(
                                    out=at[:, r : r + P],
                                    in_=at[:, r : r + P],
                                    compare_op=IS_GE,
                                    fill=0.0,
                                    base=0,
                                    pattern=[[1, P]],
                                    channel_multiplier=-1,
                                )
                            else:
                                nc.scalar.activation(at[:], sp[:], EXP, scale=0.125)
                            for qs in range(4):
                                if kc - 4 * qb > qs:
                                    continue  # fully masked sub-block
                                # one accumulation group per og bank: the first
                                # matmul's start clears has_written for the
                                # whole bank; later matmuls overwrite where
                                # unwritten / accumulate where written
                                nc.tensor.matmul(
                                    og[g][:, qs, :],
                                    lhsT=at[:, qs * P : (qs + 1) * P],
                                    rhs=vS[:, kc, strip, :],
                                    start=(kc == 0 and qs == 0),
                                    stop=(kc == 4 * qb + 3 and qs == 3),
                                )
                    # normalize rows, combine groups: o = c1*o1/s1 - c2*o2/s2
                    rc = [nrm_pool.tile([P, 4, 1], F32, tag="rc", name="rc") for _ in range(NG)]
                    for g in range(NG):
                        nc.vector.reciprocal(rc[g][:], og[g][:, :, DH : DH + 1])
                        nc.vector.tensor_scalar_mul(
                            rc[g][:], rc[g][:], c1 if g == 0 else -c2
                        )
                    t0 = nrm_pool.tile([P, 4, DH], F32, tag="tt")
                    t1 = nrm_pool.tile([P, 4, DH], F32, tag="tt")
                    nc.vector.tensor_tensor(
                        t0[:], og[0][:, :, 0:DH], rc[0][:].to_broadcast([P, 4, DH]), MULT
                    )
                    nc.vector.tensor_tensor(
                        t1[:], og[1][:, :, 0:DH], rc[1][:].to_broadcast([P, 4, DH]), MULT
                    )
                    nc.vector.tensor_add(
                        o_d[:, qb * 4 : qb * 4 + 4, hh * DH : (hh + 1) * DH],
                        t0[:],
                        t1[:],
                    )

        # ---- phase 3: o^T, out = o @ Wo ----
        with ExitStack() as ph3:
            odT_pool = ph3.enter_context(tc.tile_pool(name="odT", bufs=1))
            odT = odT_pool.tile([P, OROWS // P, S], F32R)
            wos = odT_pool.tile([P, OROWS // P, D], F32R)
            wo_st_pool = ph3.enter_context(tc.tile_pool(name="wost", bufs=2))
            for mc in range(OROWS // P):
                wst = wo_st_pool.tile([P, D], F32, tag="wost", name="wost")
                nc.sync.dma_start(wst[:], wo[mc * P : (mc + 1) * P, :])
                nc.vector.tensor_copy(wos[:, mc, :], wst[:])
            tp2 = ph3.enter_context(tc.tile_pool(name="tp2", bufs=4, space="PSUM"))
            for t in range(TOKC):
                for mc in range(OROWS // P):
                    pt = tp2.tile([P, P], F32, tag="tp2")
                    nc.tensor.transpose(
                        pt[:], o_d[:, t, mc * P : (mc + 1) * P], ident[:]
                    )
                    nc.vector.tensor_copy(odT[:, mc, t * P : (t + 1) * P], pt[:])
            out_psum = ph3.enter_context(tc.tile_pool(name="outp", bufs=4, space="PSUM"))
            outs_pool = ph3.enter_context(tc.tile_pool(name="outs", bufs=3))
            for t in range(TOKC):
                for nb in range(D // QB):
                    op = out_psum.tile([P, QB], F32, tag="op")
                    for mc in range(OROWS // P):
                        nc.tensor.matmul(
                            op[:],
                            lhsT=odT[:, mc, t * P : (t + 1) * P],
                            rhs=wos[:, mc, nb * QB : (nb + 1) * QB],
                            start=(mc == 0),
                            stop=(mc == OROWS // P - 1),
                        )
                    ot = outs_pool.tile([P, QB], F32, tag="ot")
                    nc.vector.tensor_copy(ot[:], op[:])
                    nc.sync.dma_start(
                        out[t * P : (t + 1) * P, nb * QB : (nb + 1) * QB], ot[:]
                    )

    return nc


_PROGRAM_CACHE: dict = {}


def _get_program(c1: float, c2: float) -> bass.Bass:
    key = (round(c1, 12), round(c2, 12))
    if key not in _PROGRAM_CACHE:
        _PROGRAM_CACHE[key] = build_program(c1, c2)
    return _PROGRAM_CACHE[key]


def make_in_maps(x, Wq, Wk, Wv, Wo):
    """Shard full inputs into the 8 per-core input dicts."""
    x = np.asarray(x, np.float32)
    in_maps = []
    for c in range(NCORES):
        b, qd = divmod(c, 4)
        cols = np.concatenate(
            [
                np.arange(DH) + g * (NH_TOT * DH) + (4 * qd + hh) * DH
                for hh in range(NHC)
                for g in range(NG)
            ]
        )
        in_maps.append(
            {
                "xb": np.ascontiguousarray(x[b]),
                "wq": np.ascontiguousarray(np.asarray(Wq, np.float32)[:, cols]),
                "wk": np.ascontiguousarray(np.asarray(Wk, np.float32)[:, cols]),
                "wv": np.ascontiguousarray(np.asarray(Wv, np.float32)[:, cols]),
                "wo": np.ascontiguousarray(
                    np.asarray(Wo, np.float32)[qd * OROWS : (qd + 1) * OROWS, :]
                ),
            }
        )
    return in_maps


def kernel(x, Wq, Wk, Wv, Wo, lq1, lk1, lq2, lk2):
    global LAST_RESULT
    lam = float(
        np.exp(np.float32(np.dot(lq1, lk1)))
        - np.exp(np.float32(np.dot(lq2, lk2)))
        + np.float32(LAMBDA_INIT)
    )
    c1 = 1.0 - LAMBDA_INIT
    c2 = (1.0 - LAMBDA_INIT) * lam
    nc = _get_program(c1, c2)
    in_maps = make_in_maps(x, Wq, Wk, Wv, Wo)
    res = run_bass_kernel_spmd(nc, in_maps, list(range(NCORES)))
    LAST_RESULT = res
    B = 2
    out64 = np.zeros((B, S, D), np.float64)
    for c in range(NCORES):
        out64[c // 4] += res.results[c]["out"].astype(np.float64)
    return out64.astype(np.float32)
